# revision 19
# baseline (speedup 1.0000x reference)
"""Trainium2 Bass kernel for nn_COMET_26439818674626 (ragged_sequence).

Contract: kernel(**inputs) takes FULL unsharded inputs (numpy), returns the
full outputs (y_hat, Q_sub, w_sub, confidence) matching reference.py.

Strategy: data-parallel over B across 8 NeuronCores (8 samples each);
weights/codebook replicated. Math is restructured (validated exactly
equivalent to the reference):
  - No ragged compaction: attention over all N variates with key-mask bias
    injected into scores; Q_sub is a masked mean over observed queries, so
    permutation is irrelevant and per-query attention outputs are never
    materialized (only masked column/row sums).
  - h [B,N,L,D] is never materialized: only h_mean (mean over patches) is
    needed downstream; the patch loop accumulates W_t2-projected gelu
    activations directly in PSUM.
  - E_restored collapses: y = h_mean @ W_head + (1-mask)*alpha*(fill@W_head).
"""
import sys
import numpy as np

for _p in ("/opt/trn_rl_repo", "/root/.axon_site", "/root/.axon_site/_ro/trn_rl_repo"):
    if _p not in sys.path:
        sys.path.append(_p)

import concourse.bass as bass
import concourse.bacc as bacc
import concourse.mybir as mybir
import concourse.tile as tile

F32 = mybir.dt.float32
F32R = mybir.dt.float32r
BF16 = mybir.dt.bfloat16
AF = mybir.ActivationFunctionType
AX = mybir.AxisListType

B, N, T = 64, 256, 96
P, S, D, H, K, PL = 8, 4, 128, 8, 16, 24
L = (T - P) // S + 1          # 23
TAU, RALPHA = 0.5, 0.1
NCORES = 8
BSH = B // NCORES             # 8 samples per core
BN = BSH * N                  # 2048 rows per core
NCH = 4                       # MLP bn-chunks
CHW = BN // NCH               # 512 chunk width
dh = D // H                   # 16

USE_F32R = True               # heavy matmuls in float32r (PE 4x faster)
GELU_FUNC = AF.Gelu_apprx_tanh  # matches jax.nn.gelu(approximate=True)

MDT = F32R if USE_F32R else F32


def build_nc(gelu_func=None, mdt=None, debug=False):
    """Build the per-core Bass module (SPMD: all cores run the same program)."""
    gelu_func = GELU_FUNC if gelu_func is None else gelu_func
    mdt = MDT if mdt is None else mdt
    nc = bacc.Bacc(trn_type="TRN2")

    # ---- I/O ----
    x_d = nc.dram_tensor("x", [BN, T], F32, kind="ExternalInput").ap()
    mask_d = nc.dram_tensor("mask_f", [BSH, N], F32, kind="ExternalInput").ap()
    G_d = nc.dram_tensor("G", [T + 1, L * 2 * D], F32, kind="ExternalInput").ap()
    Gm_d = nc.dram_tensor("Gmean", [T + 1, D], F32, kind="ExternalInput").ap()
    Wt2_d = nc.dram_tensor("Wt2L", [D, 2 * D], F32, kind="ExternalInput").ap()
    VET_d = nc.dram_tensor("VET", [D, N], F32, kind="ExternalInput").ap()
    Wq_d = nc.dram_tensor("Wq32", [D, 2 * D], F32, kind="ExternalInput").ap()
    Wk_d = nc.dram_tensor("Wk32", [D, 2 * D], F32, kind="ExternalInput").ap()
    Wv_d = nc.dram_tensor("Wv", [D, D], F32, kind="ExternalInput").ap()
    Wo_d = nc.dram_tensor("Wo", [D, D], F32, kind="ExternalInput").ap()
    Wdec_d = nc.dram_tensor("Wdec", [D, D], F32, kind="ExternalInput").ap()
    Whead_d = nc.dram_tensor("Whead", [D, PL], F32, kind="ExternalInput").ap()
    C_d = nc.dram_tensor("C", [K, D], F32, kind="ExternalInput").ap()
    CT_d = nc.dram_tensor("CT", [D, K], F32, kind="ExternalInput").ap()
    id_d = nc.dram_tensor("ident", [D, D], F32, kind="ExternalInput").ap()
    ones_d = nc.dram_tensor("ones_r", [1, 512], F32, kind="ExternalInput").ap()

    y_d = nc.dram_tensor("y", [BN, PL], F32, kind="ExternalOutput").ap()
    qsub_d = nc.dram_tensor("qsub", [BSH, D], F32, kind="ExternalOutput").ap()
    wsub_d = nc.dram_tensor("wsub", [BSH, K], F32, kind="ExternalOutput").ap()
    conf_d = nc.dram_tensor("conf", [BSH, 1], F32, kind="ExternalOutput").ap()
    DBG = {}
    if debug:
        for nm, shp in [("d_hmean", [D, N]), ("d_tokens", [D, N]),
                        ("d_q32", [D, N]), ("d_k32", [D, N]),
                        ("d_kexp", [128, 4 * N]), ("d_psc", [128, 4 * N]),
                        ("d_e", [128, H * N]), ("d_r", [128, BSH * 2 * H]),
                        ("d_w", [128, H]), ("d_c", [1, H * N]),
                        ("d_ccols", [128, BSH * 2 * H]), ("d_avT", [128, BSH]),
                        ("d_qpre", [D, BSH]), ("d_tokT", [128, N]),
                        ("d_vT", [128, N]), ("d_maskT2", [128, 2 * BSH]),
                        ("d_maskTq", [128, 2 * BSH]),
                        ("d_memb", [128, 2 * BSH * BSH]), ("d_mbt", [1, BSH * N])]:
            DBG[nm] = nc.dram_tensor(nm, shp, F32, kind="ExternalOutput").ap()

    def bAP(t, off_elems, pairs):
        return bass.AP(tensor=t.tensor, offset=t.offset + off_elems, ap=pairs)

    with tile.TileContext(nc) as tc:
        from contextlib import ExitStack
        ctx = ExitStack()
        with ctx:
            per = ctx.enter_context(tc.tile_pool(name="persist", bufs=1))
            dram = ctx.enter_context(tc.tile_pool(name="scratch", bufs=1, space="DRAM"))

            # ---- persistent SBUF loads ----
            # f32r tensors are staged through F32 DMAs + DVE converting
            # copies: direct f32r DMAs corrupt concurrent DMA traffic.
            G_sb = per.tile([T + 1, L * 2 * D], mdt, name="G_sb")
            Gm_sb = per.tile([T + 1, D], mdt, name="Gm_sb")
            Wt2_sb = per.tile([D, 2 * D], mdt, name="Wt2_sb")
            with tc.tile_pool(name="stage", bufs=1) as stage:
                G_st = stage.tile([T + 1, L * 2 * D], F32, name="G_st")
                nc.sync.dma_start(out=G_st, in_=G_d)
                Gm_st = stage.tile([T + 1, D], F32, name="Gm_st")
                nc.sync.dma_start(out=Gm_st, in_=Gm_d)
                Wt2_st = stage.tile([D, 2 * D], F32, name="Wt2_st")
                nc.sync.dma_start(out=Wt2_st, in_=Wt2_d)
                with nc.allow_low_precision(reason="f32r weights"):
                    nc.vector.tensor_copy(G_sb, G_st)
                    nc.vector.tensor_copy(Gm_sb, Gm_st)
                    nc.vector.tensor_copy(Wt2_sb, Wt2_st)
            VET_sb = per.tile([D, N], F32, name="VET_sb")
            nc.sync.dma_start(out=VET_sb, in_=VET_d)
            Wq_sb = per.tile([D, 2 * D], F32, name="Wq_sb")
            nc.sync.dma_start(out=Wq_sb, in_=Wq_d)
            Wk_sb = per.tile([D, 2 * D], F32, name="Wk_sb")
            nc.sync.dma_start(out=Wk_sb, in_=Wk_d)
            Wv_sb = per.tile([D, D], F32, name="Wv_sb")
            nc.sync.dma_start(out=Wv_sb, in_=Wv_d)
            Wo_sb = per.tile([D, D], F32, name="Wo_sb")
            nc.sync.dma_start(out=Wo_sb, in_=Wo_d)
            Wdec_sb = per.tile([D, D], F32, name="Wdec_sb")
            nc.sync.dma_start(out=Wdec_sb, in_=Wdec_d)
            Whead_sb = per.tile([D, PL], F32, name="Whead_sb")
            nc.sync.dma_start(out=Whead_sb, in_=Whead_d)
            C_sb = per.tile([K, D], F32, name="C_sb")
            nc.sync.dma_start(out=C_sb, in_=C_d)
            CT_sb = per.tile([D, K], F32, name="CT_sb")
            nc.sync.dma_start(out=CT_sb, in_=CT_d)
            id_sb = per.tile([D, D], F32, name="id_sb")
            nc.sync.dma_start(out=id_sb, in_=id_d)
            mask_sb = per.tile([BSH, N], F32, name="mask_sb")
            nc.sync.dma_start(out=mask_sb, in_=mask_d)

            # mask-derived quantities
            nobs = per.tile([BSH, 1], F32, name="nobs")
            nc.vector.reduce_sum(out=nobs, in_=mask_sb, axis=AX.X)
            inv_nobs = per.tile([BSH, 1], F32, name="inv_nobs")
            nc.vector.reciprocal(inv_nobs, nobs)
            maskbias = per.tile([BSH, N], F32, name="maskbias")
            nc.vector.tensor_scalar(out=maskbias, in0=mask_sb, scalar1=1.0,
                                    scalar2=1e9,
                                    op0=mybir.AluOpType.subtract,
                                    op1=mybir.AluOpType.mult)

            # DRAM bounces of mask data
            mb_dram = dram.tile([BSH, N], F32, name="mb_dram")
            nc.sync.dma_start(out=mb_dram, in_=maskbias)
            msk_dram = dram.tile([BSH, N], F32, name="msk_dram")
            nc.sync.dma_start(out=msk_dram, in_=mask_sb)

            # mb_t [1, BSH*N]: maskbias rows flattened (staged to f32r)
            mb_tf = per.tile([1, BSH * N], F32, name="mb_tf")
            nc.sync.dma_start(out=mb_tf, in_=bAP(mb_dram, 0, [[1, BSH * N]]))
            mb_t = per.tile([1, BSH * N], BF16, name="mb_t")
            with nc.allow_low_precision(reason="bf16 maskbias"):
                nc.vector.tensor_copy(mb_t, mb_tf)
            # mask columns per bn-chunk [128, 16] (for x zero-fill)
            maskcols = per.tile([128, BN // 128], F32, name="maskcols")
            nc.sync.dma_start(out=maskcols,
                              in_=bAP(msk_dram, 0, [[1, 128], [128, BN // 128]]))
            # maskT2 [128, (nh, b)]: mask as columns per n-half (tokensum rhs src)
            maskT2 = per.tile([128, 2 * BSH], F32, name="maskT2")
            for nh in range(2):
                nc.sync.dma_start(
                    out=bAP(maskT2, nh * BSH, [maskT2.ap[0], [1, BSH]]),
                    in_=bAP(msk_dram, nh * 128, [[1, 128], [N, BSH]]))
            # maskTq [128, (b, qh)]: query-mask columns
            maskTq = per.tile([128, BSH * 2], F32, name="maskTq")
            for qh in range(2):
                nc.sync.dma_start(
                    out=bAP(maskTq, qh, [maskTq.ap[0], [2, BSH]]),
                    in_=bAP(msk_dram, qh * 128, [[1, 128], [N, BSH]]))
            # one_minus_mask flat [1, BN]
            omm = per.tile([1, BN], F32, name="omm")
            nc.sync.dma_start(out=omm, in_=bAP(msk_dram, 0, [[1, BN]]))
            nc.vector.tensor_scalar(out=omm, in0=omm, scalar1=-1.0,
                                    scalar2=1.0, op0=mybir.AluOpType.mult,
                                    op1=mybir.AluOpType.add)

            # mask_emb [128, (nh, b, b')]: block-embedded mask cols for tokensum
            mask_emb = per.tile([128, 2 * BSH * BSH], F32, name="mask_emb")
            nc.vector.memset(mask_emb, 0.0)
            me3 = mask_emb.rearrange("p (nh b bp) -> p nh b bp", nh=2, b=BSH)
            mt3 = maskT2.rearrange("p (nh b) -> p nh b", nh=2)
            for nh in range(2):
                for b in range(BSH):
                    nc.vector.tensor_copy(me3[:, nh, b, b:b + 1], mt3[:, nh, b:b + 1])

            ones_f = per.tile([1, 512], F32, name="ones_f")
            nc.sync.dma_start(out=ones_f, in_=ones_d)
            ones_sb = per.tile([1, 512], BF16, name="ones_sb")
            zrow = per.tile([1, 512], BF16, name="zrow")
            with nc.allow_low_precision(reason="bf16 consts"):
                nc.vector.tensor_copy(ones_sb, ones_f)
                nc.vector.tensor_scalar_mul(zrow, ones_f, 0.0)
            ones1 = ones_sb[0:1, 0:128]
            zcol = zrow[0:1, 0:1]

            hmean = per.tile([D, BN], F32, name="hmean")

            # ================= Stage 1: patch MLP -> h_mean =================
            with tc.tile_pool(name="mlp_ps", bufs=1, space="PSUM") as mlp_ps, \
                 tc.tile_pool(name="mlp_sb", bufs=1) as mlp_sb:
                for c in range(NCH):
                    xT = mlp_sb.tile([T + 1, CHW], mdt, name="xT", tag="xT", bufs=2)
                    for j in range(4):
                        ch = c * 4 + j
                        xt_in = mlp_sb.tile([128, T], F32, name="xt_in",
                                            tag="xt_in", bufs=3)
                        nc.sync.dma_start(out=xt_in, in_=x_d[ch * 128:(ch + 1) * 128, :])
                        nc.vector.tensor_scalar_mul(xt_in, xt_in,
                                                    maskcols[:, ch:ch + 1])
                        pxt = mlp_ps.tile([T, 128], F32, name="pxt", tag="pxt", bufs=2)
                        nc.tensor.transpose(pxt, xt_in, id_sb)
                        with nc.allow_low_precision(reason="f32r xT"):
                            nc.vector.tensor_copy(
                                xT[0:T, j * 128:(j + 1) * 128], pxt)
                    with nc.allow_low_precision(reason="f32r xT ones"):
                        nc.vector.tensor_copy(xT[T:T + 1, :], ones_f)

                    pv = mlp_ps.tile([D, CHW], F32, name="pv", tag="pv", bufs=2)
                    for l in range(L):
                        pu = mlp_ps.tile([128, 2 * CHW], F32, name="pu",
                                         tag="pu", bufs=2)
                        for mh in range(2):
                            nc.tensor.matmul(
                                pu[:, mh * CHW:(mh + 1) * CHW],
                                G_sb[:, l * 2 * D + mh * D: l * 2 * D + (mh + 1) * D],
                                xT, start=True, stop=True)
                        gl = mlp_sb.tile([128, 2 * CHW], mdt, name="gl",
                                         tag="gl", bufs=2)
                        with nc.allow_low_precision(reason="f32r gelu out"):
                            nc.scalar.activation(gl, pu, gelu_func)
                        for kh in range(2):
                            nc.tensor.matmul(
                                pv, Wt2_sb[:, kh * D:(kh + 1) * D],
                                gl[:, kh * CHW:(kh + 1) * CHW],
                                start=(l == 0 and kh == 0), stop=False)
                    nc.tensor.matmul(pv, Gm_sb, xT, start=False, stop=True)
                    nc.vector.tensor_copy(hmean[:, c * CHW:(c + 1) * CHW], pv)

            # ================= Stage 2: tokens + projections =================
            tokens = per.tile([D, BN], F32, name="tokens")
            for b in range(BSH):
                nc.vector.tensor_add(tokens[:, b * N:(b + 1) * N],
                                     hmean[:, b * N:(b + 1) * N], VET_sb)

            if debug:
                nc.sync.dma_start(out=DBG["d_hmean"], in_=hmean[:, 0:N])
                nc.sync.dma_start(out=DBG["d_tokens"], in_=tokens[:, 0:N])
                nc.sync.dma_start(out=DBG["d_maskT2"], in_=maskT2)
                nc.sync.dma_start(out=DBG["d_maskTq"], in_=maskTq)
                nc.sync.dma_start(out=DBG["d_memb"], in_=mask_emb)
                dbg_mbt = per.tile([1, BSH * N], F32, name="dbg_mbt")
                nc.vector.tensor_copy(dbg_mbt, mb_t)
                nc.sync.dma_start(out=DBG["d_mbt"], in_=dbg_mbt)
            tokens_r = per.tile([D, BN], mdt, name="tokens_r")
            Wqr = per.tile([D, 2 * D], mdt, name="Wqr")
            Wkr = per.tile([D, 2 * D], mdt, name="Wkr")
            with nc.allow_low_precision(reason="f32r proj"):
                nc.vector.tensor_copy(tokens_r, tokens)
                nc.vector.tensor_copy(Wqr, Wq_sb)
                nc.vector.tensor_copy(Wkr, Wk_sb)
            q32 = per.tile([D, 2 * BN], BF16, name="q32")
            k32 = per.tile([D, 2 * BN], BF16, name="k32")
            vT = per.tile([128, BN // 128 * D], F32, name="vT")
            tokT = per.tile([128, BN // 128 * D], F32, name="tokT")
            with tc.tile_pool(name="proj_ps", bufs=1, space="PSUM") as proj_ps:
                for grp in range(2):
                    for c4 in range(NCH):
                        pp = proj_ps.tile([128, CHW], F32, name="pp", tag="pp", bufs=3)
                        nc.tensor.matmul(pp, Wqr[:, grp * D:(grp + 1) * D],
                                         tokens_r[:, c4 * CHW:(c4 + 1) * CHW],
                                         start=True, stop=True)
                        with nc.allow_low_precision(reason="bf16 q"):
                            nc.vector.tensor_copy(
                                q32[:, grp * BN + c4 * CHW:
                                    grp * BN + (c4 + 1) * CHW], pp)
                        pp2 = proj_ps.tile([128, CHW], F32, name="pp2",
                                           tag="pp", bufs=3)
                        nc.tensor.matmul(pp2, Wkr[:, grp * D:(grp + 1) * D],
                                         tokens_r[:, c4 * CHW:(c4 + 1) * CHW],
                                         start=True, stop=True)
                        with nc.allow_low_precision(reason="bf16 k"):
                            nc.vector.tensor_copy(
                                k32[:, grp * BN + c4 * CHW:
                                    grp * BN + (c4 + 1) * CHW], pp2)
                for ncH in range(BN // 128):
                    pvv = proj_ps.tile([128, D], F32, name="pvv", tag="pvv", bufs=3)
                    nc.tensor.matmul(pvv, tokens[:, ncH * 128:(ncH + 1) * 128],
                                     Wv_sb, start=True, stop=True)
                    nc.vector.tensor_copy(vT[:, ncH * D:(ncH + 1) * D], pvv)
                    ptt = proj_ps.tile([128, D], F32, name="ptt", tag="pvv", bufs=3)
                    nc.tensor.transpose(ptt, tokens[:, ncH * 128:(ncH + 1) * 128],
                                        id_sb)
                    nc.vector.tensor_copy(tokT[:, ncH * D:(ncH + 1) * D], ptt)

            if debug:
                dbg_q = per.tile([D, N], F32, name="dbg_q")
                nc.vector.tensor_copy(dbg_q, q32[:, 0:N])
                nc.sync.dma_start(out=DBG["d_q32"], in_=dbg_q)
                dbg_k = per.tile([D, N], F32, name="dbg_k")
                nc.vector.tensor_copy(dbg_k, k32[:, 0:N])
                nc.sync.dma_start(out=DBG["d_k32"], in_=dbg_k)
                nc.sync.dma_start(out=DBG["d_tokT"], in_=tokT[:, 0:N])
                nc.sync.dma_start(out=DBG["d_vT"], in_=vT[:, 0:N])

            # ================= Stage 3: attention =================
            r_sb = per.tile([128, BSH * 2 * H], F32, name="r_sb")
            c_dram = dram.tile([BSH * 2 * H, 128], F32, name="c_dram")
            with tc.tile_pool(name="att_ps", bufs=1, space="PSUM") as att_ps, \
                 tc.tile_pool(name="att_sb", bufs=1) as att_sb:
                for b in range(BSH):
                    # Kexp per head-group: block-diagonal expanded keys
                    kexps = []
                    for grp in range(2):
                        kexp = att_sb.tile([128, 4 * N], BF16, name="kexp",
                                           tag="kexp", bufs=3)
                        with nc.allow_low_precision(reason="bf16 kexp"):
                            nc.vector.memset(kexp, 0.0)
                            for j in range(4):
                                nc.vector.tensor_copy(
                                    kexp[32 * j:32 * j + 32, j * N:(j + 1) * N],
                                    k32[32 * j:32 * j + 32,
                                        grp * BN + b * N: grp * BN + (b + 1) * N])
                        if debug and b == 0 and grp == 0:
                            dbg_ke = att_sb.tile([128, 4 * N], F32, name="dbg_ke")
                            nc.vector.tensor_copy(dbg_ke, kexp)
                            nc.sync.dma_start(out=DBG["d_kexp"], in_=dbg_ke)
                        kexps.append(kexp)
                    pc = att_ps.tile([1, H * N], F32, name="pc", tag="pc", bufs=1)
                    for qh in range(2):
                        e_t = att_sb.tile([128, H * N], mdt, name="e_t",
                                          tag="e_t", bufs=2)
                        for half in range(2):   # two [128, 1024] score tiles
                            psc = att_ps.tile([128, 4 * N], F32, name="psc",
                                              tag="psc", bufs=2)
                            # mask-bias injection first (start per bank)
                            for hh in range(4):
                                nc.tensor.matmul(
                                    psc[:, hh * N:(hh + 1) * N], ones1,
                                    mb_t[0:1, b * N:(b + 1) * N],
                                    start=(hh % 2 == 0), stop=False)
                            for i in range(2):
                                nc.tensor.matmul(
                                    psc[:, i * 512:(i + 1) * 512],
                                    q32[:, half * BN + b * N + qh * 128:
                                        half * BN + b * N + (qh + 1) * 128],
                                    kexps[half][:, i * 512:(i + 1) * 512],
                                    start=False, stop=True)
                            if debug and b == 0 and qh == 0 and half == 0:
                                dbg_ps = att_sb.tile([128, 4 * N], F32,
                                                     name="dbg_ps")
                                nc.vector.tensor_copy(dbg_ps, psc)
                                nc.sync.dma_start(out=DBG["d_psc"], in_=dbg_ps)
                            with nc.allow_low_precision(reason="f32r exp out"):
                                nc.scalar.activation(
                                    e_t[:, half * 4 * N:(half + 1) * 4 * N],
                                    psc, AF.Exp, scale=float(dh) ** -0.5)
                        # r = row sums per head
                        rcol = (b * 2 + qh) * H
                        nc.vector.reduce_sum(
                            out=r_sb[:, rcol:rcol + H],
                            in_=e_t.bitcast(F32).rearrange("p (h k) -> p h k", h=H),
                            axis=AX.X)
                        # w = mask_q / r
                        w_sb = att_sb.tile([128, H], mdt, name="w_sb",
                                           tag="w_sb", bufs=2)
                        mq = maskTq
                        mq_b = bass.AP(tensor=mq.tensor,
                                       offset=mq.offset + (b * 2 + qh),
                                       ap=[mq.ap[0], [0, H]])
                        with nc.allow_low_precision(reason="f32r softmax w"):
                            nc.vector.reciprocal(w_sb, r_sb[:, rcol:rcol + H])
                            nc.vector.tensor_mul(w_sb, w_sb, mq_b)
                        if debug and b == 0 and qh == 0:
                            dbg_e = att_sb.tile([128, H * N], F32, name="dbg_e")
                            nc.vector.tensor_copy(dbg_e, e_t.bitcast(F32))
                            nc.sync.dma_start(out=DBG["d_e"], in_=dbg_e)
                            dbg_w = att_sb.tile([128, H], F32, name="dbg_w")
                            nc.vector.tensor_copy(dbg_w, w_sb.bitcast(F32))
                            nc.sync.dma_start(out=DBG["d_w"], in_=dbg_w)
                        # c-mms: c[h, k] = sum_q w[q,h] e[q, h*N+k]
                        for h in range(H):
                            jj, hi = h % 4, h // 4
                            nc.tensor.matmul(
                                pc[0:1, h * N:(h + 1) * N],
                                w_sb[:, h:h + 1], e_t[:, h * N:(h + 1) * N],
                                start=(qh == 0 and h % 2 == 0),
                                stop=(qh == 1 and h % 2 == 1))
                    c_sb = att_sb.tile([1, H * N], F32, name="c_sb",
                                       tag="c_sb", bufs=2)
                    nc.vector.tensor_copy(c_sb, pc)
                    if debug and b == 0:
                        nc.sync.dma_start(out=DBG["d_c"], in_=c_sb)
                    for kh in range(2):
                        nc.sync.dma_start(
                            out=bAP(c_dram, (b * 2 + kh) * H * 128, [[1, H * 128]]),
                            in_=bAP(c_sb, kh * 128,
                                    [c_sb.ap[0], [N, H], [1, 128]]))

            # ================= Stage 4: sum-AV + Q_sub =================
            with tc.tile_pool(name="tail_ps", bufs=1, space="PSUM") as tail_ps, \
                 tc.tile_pool(name="tail_sb", bufs=1) as tail_sb:
                c_cols = tail_sb.tile([128, BSH * 2 * H], F32, name="c_cols")
                nc.sync.dma_start(out=c_cols,
                                  in_=bAP(c_dram, 0, [[1, 128], [128, BSH * 2 * H]]))
                av_sb = tail_sb.tile([1, BSH * D], F32, name="av_sb")
                for b in range(BSH):
                    pav = tail_ps.tile([1, D], F32, name="pav", tag="pav", bufs=2)
                    for kh in range(2):
                        for h in range(H):
                            col = b * 16 + kh * 8 + h
                            nc.tensor.matmul(
                                pav[0:1, h * dh:(h + 1) * dh],
                                c_cols[:, col:col + 1],
                                vT[:, (2 * b + kh) * D + h * dh:
                                   (2 * b + kh) * D + (h + 1) * dh],
                                start=(kh == 0 and h == 0),
                                stop=(kh == 1 and h == H - 1))
                    nc.vector.tensor_copy(av_sb[:, b * D:(b + 1) * D], pav)
                av_dram = dram.tile([1, BSH * D], F32, name="av_dram")
                nc.sync.dma_start(out=av_dram, in_=av_sb)
                avT = tail_sb.tile([128, BSH], F32, name="avT")
                nc.sync.dma_start(out=avT, in_=bAP(av_dram, 0, [[1, 128], [128, BSH]]))

                if debug:
                    nc.sync.dma_start(out=DBG["d_ccols"], in_=c_cols)
                    nc.sync.dma_start(out=DBG["d_avT"], in_=avT)
                    nc.sync.dma_start(out=DBG["d_r"], in_=r_sb)
                pq = tail_ps.tile([D, BSH], F32, name="pq", tag="pq", bufs=1)
                nc.tensor.matmul(pq, Wo_sb, avT, start=True, stop=False)
                for b in range(BSH):
                    for nh in range(2):
                        nc.tensor.matmul(
                            pq, tokT[:, (2 * b + nh) * D:(2 * b + nh + 1) * D],
                            mask_emb[:, (nh * BSH + b) * BSH:(nh * BSH + b + 1) * BSH],
                            start=False, stop=(b == BSH - 1 and nh == 1))
                qpre = tail_sb.tile([D, BSH], F32, name="qpre")
                nc.vector.tensor_copy(qpre, pq)
                if debug:
                    nc.sync.dma_start(out=DBG["d_qpre"], in_=qpre)
                pqt = tail_ps.tile([BSH, D], F32, name="pqt", tag="pav", bufs=2)
                nc.tensor.transpose(pqt, qpre, id_sb)
                qrows = tail_sb.tile([BSH, D], F32, name="qrows")
                nc.vector.tensor_copy(qrows, pqt)
                nc.vector.tensor_scalar_mul(qrows, qrows, inv_nobs)
                nc.sync.dma_start(out=qsub_d, in_=qrows)

                # ---- codebook ----
                plg = tail_ps.tile([BSH, K], F32, name="plg", tag="pav", bufs=2)
                nc.tensor.matmul(plg, qpre, CT_sb, start=True, stop=True)
                sc_col = tail_sb.tile([BSH, 1], F32, name="sc_col")
                nc.vector.tensor_scalar_mul(sc_col, inv_nobs, 1.0 / TAU)
                e2 = tail_sb.tile([BSH, K], F32, name="e2")
                r2 = tail_sb.tile([BSH, 1], F32, name="r2")
                nc.scalar.activation(e2, plg, AF.Exp, scale=sc_col, accum_out=r2)
                rr2 = tail_sb.tile([BSH, 1], F32, name="rr2")
                nc.vector.reciprocal(rr2, r2)
                wsub = tail_sb.tile([BSH, K], F32, name="wsub")
                nc.vector.tensor_scalar_mul(wsub, e2, rr2)
                nc.sync.dma_start(out=wsub_d, in_=wsub)
                mx2 = tail_sb.tile([BSH, 1], F32, name="mx2")
                nc.vector.reduce_max(out=mx2, in_=e2, axis=AX.X)
                conf = tail_sb.tile([BSH, 1], F32, name="conf")
                nc.vector.tensor_mul(conf, mx2, rr2)
                nc.vector.tensor_mul(conf, conf, nobs)
                nc.vector.tensor_scalar_mul(conf, conf, 1.0 / N)
                nc.sync.dma_start(out=conf_d, in_=conf)
                alpha = tail_sb.tile([BSH, 1], F32, name="alpha")
                nc.vector.tensor_scalar(out=alpha, in0=conf,
                                        scalar1=1.0 - RALPHA, scalar2=RALPHA,
                                        op0=mybir.AluOpType.mult,
                                        op1=mybir.AluOpType.add)

                # z_ctx, fill, head row
                pwt = tail_ps.tile([K, BSH], F32, name="pwt", tag="pav", bufs=2)
                nc.tensor.transpose(pwt, wsub, id_sb[0:BSH, 0:BSH])
                wsT = tail_sb.tile([K, BSH], F32, name="wsT")
                nc.vector.tensor_copy(wsT, pwt)
                pz = tail_ps.tile([D, BSH], F32, name="pz", tag="pav", bufs=2)
                nc.tensor.matmul(pz, C_sb, wsT, start=True, stop=True)
                zcols = tail_sb.tile([D, BSH], F32, name="zcols")
                nc.vector.tensor_copy(zcols, pz)
                pf = tail_ps.tile([D, BSH], F32, name="pf", tag="pav", bufs=2)
                nc.tensor.matmul(pf, Wdec_sb, zcols, start=True, stop=True)
                fcols = tail_sb.tile([D, BSH], F32, name="fcols")
                nc.vector.tensor_copy(fcols, pf)
                phw = tail_ps.tile([BSH, PL], F32, name="phw", tag="pav", bufs=2)
                nc.tensor.matmul(phw, fcols, Whead_sb, start=True, stop=True)
                ahw = tail_sb.tile([BSH, PL], F32, name="ahw")
                nc.vector.tensor_copy(ahw, phw)

                # bounce ahw + alpha to single-partition rows
                ahw_dram = dram.tile([BSH, PL], F32, name="ahw_dram")
                nc.sync.dma_start(out=ahw_dram, in_=ahw)
                ahw_rep = tail_sb.tile([1, BSH * PL], F32, name="ahw_rep")
                nc.sync.dma_start(out=ahw_rep, in_=bAP(ahw_dram, 0, [[1, BSH * PL]]))
                al_dram = dram.tile([BSH, 1], F32, name="al_dram")
                nc.sync.dma_start(out=al_dram, in_=alpha)
                al_row = tail_sb.tile([1, BSH], F32, name="al_row")
                nc.sync.dma_start(out=al_row, in_=bAP(al_dram, 0, [[1, BSH]]))

                # s_flat = (1 - mask) * alpha[b]
                s_flat = tail_sb.tile([1, BN], F32, name="s_flat")
                for b in range(BSH):
                    nc.vector.tensor_scalar_mul(s_flat[:, b * N:(b + 1) * N],
                                                omm[:, b * N:(b + 1) * N],
                                                al_row[0:1, b:b + 1])

                # ---- y_hat ----
                y_sb = tail_sb.tile([128, (BN // 128) * PL], F32, name="y_sb")
                for ch in range(BN // 128):
                    b = ch // 2
                    py = tail_ps.tile([128, PL], F32, name="py", tag="py", bufs=3)
                    nc.tensor.matmul(py, hmean[:, ch * 128:(ch + 1) * 128],
                                     Whead_sb, start=True, stop=False)
                    nc.tensor.matmul(py, s_flat[0:1, ch * 128:(ch + 1) * 128],
                                     ahw_rep[0:1, b * PL:(b + 1) * PL],
                                     start=False, stop=True)
                    nc.vector.tensor_copy(y_sb[:, ch * PL:(ch + 1) * PL], py)
                nc.sync.dma_start(
                    out=bAP(y_d, 0, [[PL, 128], [128 * PL, BN // 128], [1, PL]]),
                    in_=y_sb)

    nc.compile()
    return nc


def host_prep(inputs):
    """Build the per-core input maps (host-side sharding + weight repacking)."""
    x_full = np.ascontiguousarray(np.asarray(inputs["x_full"], np.float32))
    obs = np.asarray(inputs["obs_mask"])
    Wp = np.asarray(inputs["W_patch"], np.float32)
    bp = np.asarray(inputs["b_patch"], np.float32)
    W1 = np.asarray(inputs["W_t1"], np.float32)
    W2 = np.asarray(inputs["W_t2"], np.float32)
    VE = np.asarray(inputs["var_embed"], np.float32)
    Wq = np.asarray(inputs["Wq"], np.float32)
    Wk = np.asarray(inputs["Wk"], np.float32)
    Wv = np.asarray(inputs["Wv"], np.float32)
    Wo = np.asarray(inputs["Wo"], np.float32)
    C = np.asarray(inputs["C"], np.float32)
    Wd = np.asarray(inputs["W_dec"], np.float32)
    Wh = np.asarray(inputs["W_head"], np.float32)

    Wpt1 = (Wp.astype(np.float64) @ W1.astype(np.float64)).astype(np.float32)
    bW1 = (bp.astype(np.float64) @ W1.astype(np.float64)).astype(np.float32)
    G = np.zeros((T + 1, L * 2 * D), np.float32)
    for l in range(L):
        G[S * l:S * l + P, l * 2 * D:(l + 1) * 2 * D] = Wpt1
        G[T, l * 2 * D:(l + 1) * 2 * D] = bW1
    Gmean = np.zeros((T + 1, D), np.float32)
    acc = np.zeros((T, D), np.float64)
    for l in range(L):
        acc[S * l:S * l + P] += Wp
    Gmean[0:T] = (acc / L).astype(np.float32)
    Gmean[T] = bp
    Wt2L = np.zeros((D, 2 * D), np.float32)
    for kh in range(2):
        Wt2L[:, kh * D:(kh + 1) * D] = (W2[kh * D:(kh + 1) * D, :] / L)

    Wq32 = np.zeros((D, 2 * D), np.float32)
    Wk32 = np.zeros((D, 2 * D), np.float32)
    for h in range(H):
        g, j = h // 4, h % 4
        Wq32[:, g * D + j * 32: g * D + j * 32 + dh] = Wq[:, h * dh:(h + 1) * dh]
        Wk32[:, g * D + j * 32: g * D + j * 32 + dh] = Wk[:, h * dh:(h + 1) * dh]

    shared = {
        "G": G, "Gmean": Gmean, "Wt2L": Wt2L,
        "VET": np.ascontiguousarray(VE.T),
        "Wq32": Wq32, "Wk32": Wk32, "Wv": Wv, "Wo": Wo,
        "Wdec": Wd, "Whead": Wh, "C": C,
        "CT": np.ascontiguousarray(C.T),
        "ident": np.eye(D, dtype=np.float32),
        "ones_r": np.ones((1, 512), np.float32),
    }
    in_maps = []
    for core in range(NCORES):
        b0 = core * BSH
        m = dict(shared)
        m["x"] = np.ascontiguousarray(
            x_full[b0:b0 + BSH].reshape(BN, T))
        m["mask_f"] = np.ascontiguousarray(obs[b0:b0 + BSH].astype(np.float32))
        in_maps.append(m)
    return in_maps


_CACHE = {}


def kernel(**inputs):
    from concourse import bass_utils
    key = "nc"
    if key not in _CACHE:
        _CACHE[key] = build_nc()
    nc = _CACHE[key]
    in_maps = host_prep(inputs)
    res = bass_utils.run_bass_kernel_spmd(nc, in_maps, core_ids=list(range(NCORES)))
    y = np.concatenate([r["y"].reshape(BSH, N, PL) for r in res.results], axis=0)
    qs = np.concatenate([r["qsub"] for r in res.results], axis=0)
    ws = np.concatenate([r["wsub"] for r in res.results], axis=0)
    cf = np.concatenate([r["conf"][:, 0] for r in res.results], axis=0)
    return (y.astype(np.float32), qs.astype(np.float32),
            ws.astype(np.float32), cf.astype(np.float32))


if __name__ == "__main__":
    import reference as ref
    inputs = {k: np.asarray(v) for k, v in ref.setup_inputs().items()}
    outs = kernel(**inputs)
    print([o.shape for o in outs])
